# revision 50
# baseline (speedup 1.0000x reference)
"""Trainium2 Bass kernel: per-pixel channel shuffle + 3x3 conv (stride 1, pad 1).

Problem: x [32,256,56,56] f32, w [256,256,3,3] f32 (OIHW), perm [3136,256] i32;
out[b,:,h,w] = conv3x3(xs)[b,:,h,w] where xs[b,:,l] = x[b, perm[l,:], l].

Strategy (8 NeuronCores, data-parallel over batch, 4 batches/core):
  The 4 local batches form one continuous 12544-pixel stream (98 groups of
  128 px).  Per scatter call (k<=7 groups): DMA a [128, k*256] bf16 tile
  (partition = pixel-within-group, free = group-major channels), one big
  GPSIMD local_scatter applies each pixel's inverse channel permutation AND
  rearranges channels ct-major (dst index table built on host, period 49
  groups = 2 images), then one batched DMA-transpose per 128-channel ic-tile
  ([128, k*128] -> [128, k, 128]) lands [channel, pixel-run] data in a
  staging ring.  ACT (ScalarE) copies insert 57-wide row padding (56 px +
  1 zero) into per-batch image planes [128, 58*57] (+1 lead zero col) so the
  3x3 conv is pure implicit GEMM: per batch 8 row-groups (7 rows, N=398 --
  phase-aligned with 16-row scatter calls, windows never read the next
  call's rows) x 2 oc-tiles x 18 matmuls (9 taps x 2 ic-tiles) accumulate
  in PSUM; DVE evicts to a bf16 staging region; the 57->56 pad strip
  happens on the host.  Engine queues are single-pacing-domain (sync:
  xin+transposes+deferred outs, gpsimd: scatters+idx, scalar: pad-copies,
  vector: evictions) because all DMAs are framework-serialized around
  DMA-transposes and cross-engine completion semaphores post ~3.5us late;
  staged dummy matmuls keep the PE's HAM clock warm through the pipeline
  fill.  PE runs only conv matmuls (~194us roofline/core); GPSIMD ~125us.
"""

import os
import sys
import types
import numpy as np

# a previous (crashed) run can leave the NeuronCores wedged
# (NRT_EXEC_UNIT_UNRECOVERABLE); request a core reset at NRT init
os.environ.setdefault("NEURON_RT_RESET_CORES", "1")

_STATE = {}
LAST_RESULT = None

B, C, H, W = 32, 256, 56, 56
HW = H * W          # 3136
N_CORES = 8
B_LOC = B // N_CORES
GPX = B_LOC * HW    # 12544 pixels per core
NGRP = GPX // 128   # 98 groups of 128 px
PERIOD = 49         # idx table period in groups (2 images: 49*128 = 2*3136)
ROWW = 57           # padded row width (56 + 1 zero col)
REG = 1 + 58 * ROWW + 1   # per-ic-tile plane region (lead zero + 58 rows + slack)
GR = 7              # output rows per conv group (8 groups/batch: readiness
                    # thresholds 7g+8 phase-align with 16-row scatter calls)
NG = GR * ROWW - 1  # conv matmul free size = 398: a 399 window would read 1
                    # elem into row 7g+9 (false WAR with the next call's
                    # pad-copies); col 398 is a stripped pad position anyway
OSTRIDE = GR * ROWW  # output staging stride (399; last elem unwritten)

# scatter calls: (start_group, n_groups). Head split [4,3]: call 0 alone
# delivers conv group (0,0)'s 9 rows, one transpose+copy stage sooner.
CALLS = [(0, 4), (4, 3)] + [(7 + 7 * i, 7) for i in range(13)]
STG_SLOTS = 6


def _install_ntff_shim():
    # antenv.axon_hooks is absent in some images; provide it so trace=True
    # (BASS_TRACE=1) can capture NTFF profiles instead of crashing.
    name = "antenv.axon_hooks"
    if name in sys.modules:
        return
    try:
        import antenv  # noqa: F401

        m = types.ModuleType(name)
        m._hook = None
        m.set_axon_ntff_profile_hook = lambda h: setattr(m, "_hook", h)
        m.get_axon_ntff_profile_hook = lambda: m._hook
        sys.modules[name] = m
        setattr(sys.modules["antenv"], "axon_hooks", m)
        from trn_agent_boot.trn_boot import _ntff_profile_via_ctypes

        hook = _ntff_profile_via_ctypes("/opt/axon/libaxon_pjrt.so")
        if hook is not None:
            m.set_axon_ntff_profile_hook(hook)
    except Exception:
        pass


def _copy_segments():
    """Per call: list of (ct, slot_off, plane_idx, plane_off, kind, n)
    kind 'rows': full rows block (src [128, n*56] -> dst [[57,n],[1,56]])
    kind 'part': partial row of n px."""
    segs = []
    ready = []  # per call: list of (b, g) conv groups newly ready
    done_rows = 0
    issued = set()
    for m, (s, k) in enumerate(CALLS):
        p0, p1 = 128 * s, 128 * (s + k)
        cur = []
        a = p0
        while a < p1:
            r = a // 56
            row_end = 56 * (r + 1)
            bnd = min(p1, row_end)
            b_idx = r // 56
            rl = r % 56
            if a == 56 * r and bnd == row_end:
                # run of full rows: extend while same batch
                r2 = r
                while (
                    56 * (r2 + 1) <= p1
                    and (r2 // 56) == b_idx
                ):
                    r2 += 1
                n = r2 - r
                cur.append((a - p0, b_idx, 1 + (1 + rl) * ROWW, "rows", n))
                a = 56 * r2
            else:
                n = bnd - a
                cur.append((a - p0, b_idx, 1 + (1 + rl) * ROWW + (a - 56 * r), "part", n))
                a = bnd
        segs.append(cur)
        done_rows = p1 // 56
        newly = []
        for b in range(B_LOC):
            for g in range(8):
                if (b, g) in issued:
                    continue
                if 56 * b + min(GR * g + GR + 1, 56) <= done_rows:
                    issued.add((b, g))
                    newly.append((b, g))
        ready.append(newly)
    assert len(issued) == B_LOC * 8
    return segs, ready


def _build_kernel():
    import concourse.bass as bass
    import concourse.mybir as mybir
    from concourse import bacc, tile
    from contextlib import ExitStack

    F32 = mybir.dt.float32
    BF16 = mybir.dt.bfloat16
    I16 = mybir.dt.int16

    nc = bacc.Bacc("TRN2", target_bir_lowering=False, debug=False, num_devices=N_CORES)

    xt = nc.dram_tensor("xt", [GPX, C], BF16, kind="ExternalInput")
    wt = nc.dram_tensor("wt", [36, 128, 128], BF16, kind="ExternalInput")
    idxh = nc.dram_tensor("idxh", [128, 7 * 256], I16, kind="ExternalInput")
    idxs = nc.dram_tensor("idxs", [128, PERIOD * 256], I16, kind="ExternalInput")
    # padded conv output, [b][p=oc%128][(oct*8+g)*399 + r*57 + x]; the
    # 57->56 pad strip happens on the host (free) so out-DMAs stay
    # contiguous 798B-per-partition runs instead of 112B strided chunks
    out = nc.dram_tensor(
        "out", [B_LOC, 128, 16 * GR * ROWW], BF16, kind="ExternalOutput"
    )

    segs, ready = _copy_segments()

    with tile.TileContext(nc) as tc, ExitStack() as ctx:
        const = ctx.enter_context(tc.tile_pool(name="const", bufs=1))

        idxh_sb = const.tile([128, 7 * 256], I16)
        idxs_sb = const.tile([128, PERIOD * 256], I16)
        wsb = const.tile([128, 36 * 128], BF16)

        # dummy tile for PE HAM warmup; memset FIRST on the vector queue so
        # the first warmup matmuls can start right after the preamble
        dtile = const.tile([128, 256], BF16, name="dtile", tag="dtile")
        nc.vector.memset(dtile[:, :], 0.0)

        # one padded image plane per batch (2 ic-tile regions each): no
        # plane WAR at all, so pad-copies are never scheduler-deferred
        # behind PE-paced conv reads
        planes = []
        for pi in range(B_LOC):
            pl = const.tile([128, 2 * REG], BF16, name=f"pl{pi}", tag=f"pl{pi}")
            nc.vector.memset(pl[:, :], 0.0)
            planes.append(pl)

        # staging rings for transposed [channel, pixel] data, one per ic-tile
        stg = [
            const.tile([128, STG_SLOTS * 896], BF16, name=f"stg{ct}", tag=f"stg{ct}")
            for ct in range(2)
        ]

        # idx tables load on the gpsimd queue: the scatters' idx deps become
        # same-queue FIFO order (no multi-us cross-engine DMA-completion
        # semaphore latency) and the scalar queue stays pure pad-copies
        nc.gpsimd.dma_start(out=idxh_sb[:, :], in_=idxh[:, :])
        nc.gpsimd.dma_start(out=idxs_sb[:, : 25 * 256], in_=idxs[:, : 25 * 256])
        nc.gpsimd.dma_start(out=idxs_sb[:, 25 * 256 :], in_=idxs[:, 25 * 256 :])
        nc.scalar.dma_start(
            out=wsb[:, :],
            in_=bass.AP(wt, 0, [[128, 128], [128 * 128, 36], [1, 128]]),
        )

        xin_pool = ctx.enter_context(tc.tile_pool(name="xin", bufs=3))
        sout_pool = ctx.enter_context(tc.tile_pool(name="sout", bufs=5))
        osts = const.tile([128, 64 * OSTRIDE], BF16, name="osts", tag="osts")
        mpsum_pool = ctx.enter_context(tc.tile_pool(name="mpsum", bufs=6, space="PSUM"))
        dpsum_pool = ctx.enter_context(tc.tile_pool(name="dpsum", bufs=1, space="PSUM"))

        # HAM warmup: staged dummy matmuls keep the PE busy through the
        # pipeline-fill window (deps fire every ~3us) so the first real
        # matmuls run at 2.4 GHz instead of paying the 1.2 GHz cold ramp
        dps = dpsum_pool.tile([128, 256], F32, name="dps", tag="dps")

        def warm_mms(n, rhs_ap):
            for _ in range(n):
                nc.tensor.matmul(
                    dps[:, :], lhsT=dtile[:, 0:128], rhs=rhs_ap,
                    start=True, stop=True,
                )

        # Outputs stage unstripped in one bf16 SBUF region and drain as 4
        # big contiguous out-DMAs issued after the loop: every DMA is
        # framework-serialized around DMA-transposes, so a PE-paced
        # out-DMA ahead of a transpose would throttle the whole supply
        # chain to PE pace.
        # tap order per group: the (dh=2, dw=2) taps get a 397-wide window
        # (the dropped psum col 397 contribution reads the zero pad column,
        # an exact no-op) and sit mid-sequence so start/stop matmuls are
        # full-width
        TAPS = [(0, t) for t in range(9)] + [(1, 8)] + [(1, t) for t in range(8)]

        def conv_group(b, g):
            pl = planes[b]
            for oct in range(2):
                mp = mpsum_pool.tile([128, NG], F32, name="mp", tag="mp")
                for i, (ct, tap) in enumerate(TAPS):
                    dh, dw = divmod(tap, 3)
                    q0 = ct * REG + (GR * g + dh) * ROWW + dw
                    widx = (ct * 9 + tap) * 2 + oct
                    n = NG - 1 if (dh == 2 and dw == 2) else NG
                    nc.tensor.matmul(
                        mp[:, :n],
                        lhsT=wsb[:, widx * 128 : (widx + 1) * 128],
                        rhs=pl[:, q0 : q0 + n],
                        start=(i == 0),
                        stop=(i == 17),
                    )
                oi = (b * 2 + oct) * 8 + g
                # evictions (PE-paced) live alone on the vector queue so
                # they never delay scatter-paced work on other queues
                nc.vector.tensor_copy(
                    osts[:, oi * OSTRIDE : oi * OSTRIDE + NG], mp[:, :]
                )

        # xin prefetch 2 calls ahead so transposes (which wait on the
        # scatter) never block the next input load on the sync FIFO
        xins = {}

        def load_xin(m):
            if m >= len(CALLS):
                return
            s, k = CALLS[m]
            kk = k * 256
            xin = xin_pool.tile([128, 7 * 256], BF16, name="xin", tag="xin")
            nc.sync.dma_start(
                out=xin[:, :kk],
                in_=bass.AP(xt, 128 * s * C, [[C, 128], [128 * C, k], [1, C]]),
            )
            xins[m] = xin

        def do_copies(mi):
            slot_i = (mi % STG_SLOTS) * 896
            for src_off, b_idx, pl_off, kind, n in segs[mi]:
                pl = planes[b_idx]
                for ct in range(2):
                    so = slot_i + src_off
                    po = ct * REG + pl_off
                    if kind == "rows":
                        src = stg[ct][:, so : so + 56 * n].rearrange(
                            "p (r x) -> p r x", r=n
                        )
                        dst = pl[:, po : po + n * ROWW].rearrange(
                            "p (r x) -> p r x", r=n
                        )[:, :, 0:56]
                        nc.scalar.copy(dst, src)
                    else:
                        nc.scalar.copy(
                            pl[:, po : po + n], stg[ct][:, so : so + n]
                        )

        load_xin(0)
        load_xin(1)
        warm_mms(20, dtile[:, 0:256])          # fires ~preamble end
        warm_mms(12, xins[0][:, 0:256])        # fires at xin0 load done
        warm_mms(10, xins[1][:, 0:256])        # fires at xin1 load done
        for m, (s, k) in enumerate(CALLS):
            kk = k * 256
            load_xin(m + 2)
            if m == 0:
                warm_mms(10, xins[2][:, 0:256])  # fires at xin2 load done
            xin = xins.pop(m)
            if m < 2:
                iap = idxh_sb[:, s * 256 : (s + k) * 256]
            else:
                sp = (s % PERIOD) * 256
                iap = idxs_sb[:, sp : sp + kk]
            sout = sout_pool.tile([128, 7 * 256], BF16, name="sout", tag="sout")
            nc.gpsimd.local_scatter(
                out_ap=sout[:, :kk],
                data_ap=xin[:, :kk],
                idxs_ap=iap,
                channels=128,
                num_elems=kk,
                num_idxs=kk,
            )
            slot = (m % STG_SLOTS) * 896
            for ct in range(2):
                dst = stg[ct][:, slot : slot + k * 128].rearrange(
                    "p (e l) -> p e l", e=k
                )
                nc.sync.dma_start_transpose(
                    dst, sout[:, ct * k * 128 : (ct + 1) * k * 128]
                )
            if m <= 1:
                # fires when this call's scatter completes (Pool sem, fast)
                warm_mms(10 if m == 0 else 8, sout[:, 0:256])
            # pad-copies on ACT (own SBUF port; DVE contends with GPSIMD);
            # the scalar queue carries ONLY these, so they are always
            # scatter-paced -- never queued behind a PE-paced eviction
            do_copies(m)
            # ready[0]'s conv issues after iteration 1's warmup phase so the
            # HAM bridge extends right up to the first real matmul
            if m == 1:
                for (b, g) in ready[0] + ready[1]:
                    conv_group(b, g)
            elif m > 1:
                for (b, g) in ready[m]:
                    conv_group(b, g)
        for b in range(B_LOC):
            for oct in range(2):
                for h in range(2):  # halves: smaller final transfer -> the
                    # kernel-end barrier waits less on its completion
                    o0 = ((b * 2 + oct) * 8 + 4 * h) * OSTRIDE
                    d0 = (oct * 8 + 4 * h) * OSTRIDE
                    nc.sync.dma_start(
                        out=out[b, :, d0 : d0 + 4 * OSTRIDE],
                        in_=osts[:, o0 : o0 + 4 * OSTRIDE],
                    )

    nc.compile()
    return nc


def _host_prep(x, w, perm):
    import ml_dtypes

    # pixel-major bf16: [B, HW, C]
    xf = np.ascontiguousarray(
        x.reshape(B, C, HW).transpose(0, 2, 1)
    ).astype(ml_dtypes.bfloat16)

    wt = np.empty((36, 128, 128), dtype=ml_dtypes.bfloat16)
    wf = np.asarray(w, dtype=np.float32)
    for ct in range(2):
        for tap in range(9):
            kh, kw = divmod(tap, 3)
            for oct in range(2):
                i = (ct * 9 + tap) * 2 + oct
                wt[i] = wf[
                    oct * 128 : (oct + 1) * 128, ct * 128 : (ct + 1) * 128, kh, kw
                ].T.astype(ml_dtypes.bfloat16)

    iperm = np.empty((HW, C), dtype=np.int16)
    np.put_along_axis(
        iperm, perm.astype(np.int64), np.arange(C, dtype=np.int16)[None, :], axis=1
    )

    # steady idx table [128, PERIOD*256]: group g, partition p -> pixel
    # (128g+p) % HW; dst = ct_major(k=7): (ip>>7)*896 + (g%7)*128 + (ip&127)
    gg = np.arange(PERIOD)
    pp = np.arange(128)
    l = (128 * gg[:, None] + pp[None, :]) % HW          # [49, 128]
    ip = iperm[l].astype(np.int32)                      # [49, 128, 256]
    j = (gg % 7).astype(np.int32)[:, None, None]
    dst = (ip >> 7) * 896 + j * 128 + (ip & 127)
    idxs = np.ascontiguousarray(
        dst.astype(np.int16).transpose(1, 0, 2).reshape(128, PERIOD * 256)
    )

    # head table for the first 7 groups with call sizes (0,4),(4,3)
    idxh = np.empty((128, 7 * 256), dtype=np.int16)
    for (s, k) in CALLS[:2]:
        for g in range(s, s + k):
            lg = (128 * g + pp) % HW
            ipg = iperm[lg].astype(np.int32)            # [128, 256]
            d = (ipg >> 7) * (k * 128) + (g - s) * 128 + (ipg & 127)
            idxh[:, g * 256 : (g + 1) * 256] = d.astype(np.int16)

    in_maps = []
    for cidx in range(N_CORES):
        in_maps.append(
            {
                "xt": np.ascontiguousarray(
                    xf[cidx * B_LOC : (cidx + 1) * B_LOC].reshape(GPX, C)
                ),
                "wt": wt,
                "idxh": idxh,
                "idxs": idxs,
            }
        )
    return in_maps


def kernel(x, w, perm):
    global LAST_RESULT
    _install_ntff_shim()
    from concourse.bass_utils import run_bass_kernel_spmd

    x = np.asarray(x, dtype=np.float32)
    w = np.asarray(w, dtype=np.float32)
    perm = np.asarray(perm)

    if "nc" not in _STATE:
        _STATE["nc"] = _build_kernel()
    nc = _STATE["nc"]

    in_maps = _host_prep(x, w, perm)
    res = run_bass_kernel_spmd(nc, in_maps, core_ids=list(range(N_CORES)))
    LAST_RESULT = res
    outs = []
    for r in res.results:
        # [b, p, (oct*8+g)*399 + r*57 + x] -> [b, oct*128+p, g*392 + r*56 + x]
        a = np.asarray(r["out"], dtype=np.float32).reshape(
            B_LOC, 128, 2, 8, GR, ROWW
        )[..., :56]
        a = a.transpose(0, 2, 1, 3, 4, 5).reshape(B_LOC, C, H, W)
        outs.append(a)
    return np.ascontiguousarray(np.concatenate(outs, axis=0), dtype=np.float32)


# revision 52
# speedup vs baseline: 1.1806x; 1.1806x over previous
"""Trainium2 Bass kernel: per-pixel channel shuffle + 3x3 conv (stride 1, pad 1).

Problem: x [32,256,56,56] f32, w [256,256,3,3] f32 (OIHW), perm [3136,256] i32;
out[b,:,h,w] = conv3x3(xs)[b,:,h,w] where xs[b,:,l] = x[b, perm[l,:], l].

Strategy (8 NeuronCores, data-parallel over batch, 4 batches/core):
  The 4 local batches form one continuous 12544-pixel stream (98 groups of
  128 px).  Per scatter call (k<=7 groups): DMA a [128, k*256] bf16 tile
  (partition = pixel-within-group, free = group-major channels), one big
  GPSIMD local_scatter applies each pixel's inverse channel permutation AND
  rearranges channels ct-major (dst index table built on host, period 49
  groups = 2 images), then one batched DMA-transpose per 128-channel ic-tile
  ([128, k*128] -> [128, k, 128]) lands [channel, pixel-run] data in a
  staging ring.  ACT (ScalarE) copies insert 57-wide row padding (56 px +
  1 zero) into per-batch image planes [128, 58*57] (+1 lead zero col) so the
  3x3 conv is pure implicit GEMM: per batch 8 row-groups (7 rows, N=398 --
  phase-aligned with 16-row scatter calls, windows never read the next
  call's rows) x 2 oc-tiles x 18 matmuls (9 taps x 2 ic-tiles) accumulate
  in PSUM; DVE evicts to a bf16 staging region; the 57->56 pad strip
  happens on the host.  Engine queues are single-pacing-domain (sync:
  xin+transposes+deferred outs, gpsimd: scatters+idx, scalar: pad-copies,
  vector: evictions) because all DMAs are framework-serialized around
  DMA-transposes and cross-engine completion semaphores post ~3.5us late;
  staged dummy matmuls keep the PE's HAM clock warm through the pipeline
  fill.  PE runs only conv matmuls (~194us roofline/core); GPSIMD ~125us.
"""

import os
import sys
import types
import numpy as np

# a previous (crashed) run can leave the NeuronCores wedged
# (NRT_EXEC_UNIT_UNRECOVERABLE); request a core reset at NRT init
os.environ.setdefault("NEURON_RT_RESET_CORES", "1")

_STATE = {}
LAST_RESULT = None

B, C, H, W = 32, 256, 56, 56
HW = H * W          # 3136
N_CORES = 8
B_LOC = B // N_CORES
GPX = B_LOC * HW    # 12544 pixels per core
NGRP = GPX // 128   # 98 groups of 128 px
PERIOD = 49         # idx table period in groups (2 images: 49*128 = 2*3136)
ROWW = 57           # padded row width (56 + 1 zero col)
REG = 1 + 58 * ROWW + 1   # per-ic-tile plane region (lead zero + 58 rows + slack)
GR = 7              # output rows per conv group (8 groups/batch: readiness
                    # thresholds 7g+8 phase-align with 16-row scatter calls)
NG = GR * ROWW - 1  # conv matmul free size = 398: a 399 window would read 1
                    # elem into row 7g+9 (false WAR with the next call's
                    # pad-copies); col 398 is a stripped pad position anyway
OSTRIDE = GR * ROWW  # output staging stride (399; last elem unwritten)

# scatter calls: (start_group, n_groups). Head split [4,3]: call 0 alone
# delivers conv group (0,0)'s 9 rows, one transpose+copy stage sooner.
CALLS = [(0, 4), (4, 3)] + [(7 + 7 * i, 7) for i in range(13)]
STG_SLOTS = 6


def _install_ntff_shim():
    # antenv.axon_hooks is absent in some images; provide it so trace=True
    # (BASS_TRACE=1) can capture NTFF profiles instead of crashing.
    name = "antenv.axon_hooks"
    if name in sys.modules:
        return
    try:
        import antenv  # noqa: F401

        m = types.ModuleType(name)
        m._hook = None
        m.set_axon_ntff_profile_hook = lambda h: setattr(m, "_hook", h)
        m.get_axon_ntff_profile_hook = lambda: m._hook
        sys.modules[name] = m
        setattr(sys.modules["antenv"], "axon_hooks", m)
        from trn_agent_boot.trn_boot import _ntff_profile_via_ctypes

        hook = _ntff_profile_via_ctypes("/opt/axon/libaxon_pjrt.so")
        if hook is not None:
            m.set_axon_ntff_profile_hook(hook)
    except Exception:
        pass


def _copy_segments():
    """Per call: list of (ct, slot_off, plane_idx, plane_off, kind, n)
    kind 'rows': full rows block (src [128, n*56] -> dst [[57,n],[1,56]])
    kind 'part': partial row of n px."""
    segs = []
    ready = []  # per call: list of (b, g) conv groups newly ready
    done_rows = 0
    issued = set()
    for m, (s, k) in enumerate(CALLS):
        p0, p1 = 128 * s, 128 * (s + k)
        cur = []
        a = p0
        while a < p1:
            r = a // 56
            row_end = 56 * (r + 1)
            bnd = min(p1, row_end)
            b_idx = r // 56
            rl = r % 56
            if a == 56 * r and bnd == row_end:
                # run of full rows: extend while same batch
                r2 = r
                while (
                    56 * (r2 + 1) <= p1
                    and (r2 // 56) == b_idx
                ):
                    r2 += 1
                n = r2 - r
                cur.append((a - p0, b_idx, 1 + (1 + rl) * ROWW, "rows", n))
                a = 56 * r2
            else:
                n = bnd - a
                cur.append((a - p0, b_idx, 1 + (1 + rl) * ROWW + (a - 56 * r), "part", n))
                a = bnd
        segs.append(cur)
        done_rows = p1 // 56
        newly = []
        for b in range(B_LOC):
            for g in range(8):
                if (b, g) in issued:
                    continue
                if 56 * b + min(GR * g + GR + 1, 56) <= done_rows:
                    issued.add((b, g))
                    newly.append((b, g))
        ready.append(newly)
    assert len(issued) == B_LOC * 8
    return segs, ready


def _build_kernel():
    import concourse.bass as bass
    import concourse.mybir as mybir
    from concourse import bacc, tile
    from contextlib import ExitStack

    F32 = mybir.dt.float32
    BF16 = mybir.dt.bfloat16
    I16 = mybir.dt.int16

    nc = bacc.Bacc("TRN2", target_bir_lowering=False, debug=False, num_devices=N_CORES)

    xt = nc.dram_tensor("xt", [GPX, C], BF16, kind="ExternalInput")
    wt = nc.dram_tensor("wt", [36, 128, 128], BF16, kind="ExternalInput")
    idxh = nc.dram_tensor("idxh", [128, 7 * 256], I16, kind="ExternalInput")
    idxs = nc.dram_tensor("idxs", [128, PERIOD * 256], I16, kind="ExternalInput")
    # padded conv output, [b][p=oc%128][(oct*8+g)*399 + r*57 + x]; the
    # 57->56 pad strip happens on the host (free) so out-DMAs stay
    # contiguous 798B-per-partition runs instead of 112B strided chunks
    out = nc.dram_tensor(
        "out", [B_LOC, 128, 16 * GR * ROWW], BF16, kind="ExternalOutput"
    )

    segs, ready = _copy_segments()

    with tile.TileContext(nc) as tc, ExitStack() as ctx:
        const = ctx.enter_context(tc.tile_pool(name="const", bufs=1))

        idxh_sb = const.tile([128, 7 * 256], I16)
        idxs_sb = const.tile([128, PERIOD * 256], I16)
        wsb = const.tile([128, 36 * 128], BF16)

        # dummy tile for PE HAM warmup; memset FIRST on the vector queue so
        # the first warmup matmuls can start right after the preamble
        dtile = const.tile([128, 256], BF16, name="dtile", tag="dtile")
        nc.vector.memset(dtile[:, :], 0.0)

        # one padded image plane per batch (2 ic-tile regions each): no
        # plane WAR at all, so pad-copies are never scheduler-deferred
        # behind PE-paced conv reads
        planes = []
        for pi in range(B_LOC):
            pl = const.tile([128, 2 * REG], BF16, name=f"pl{pi}", tag=f"pl{pi}")
            nc.vector.memset(pl[:, :], 0.0)
            planes.append(pl)

        # staging rings for transposed [channel, pixel] data, one per ic-tile
        stg = [
            const.tile([128, STG_SLOTS * 896], BF16, name=f"stg{ct}", tag=f"stg{ct}")
            for ct in range(2)
        ]

        # idx tables load on the gpsimd queue: the scatters' idx deps become
        # same-queue FIFO order (no multi-us cross-engine DMA-completion
        # semaphore latency) and the scalar queue stays pure pad-copies
        nc.gpsimd.dma_start(out=idxh_sb[:, :], in_=idxh[:, :])
        nc.gpsimd.dma_start(out=idxs_sb[:, : 25 * 256], in_=idxs[:, : 25 * 256])
        nc.gpsimd.dma_start(out=idxs_sb[:, 25 * 256 :], in_=idxs[:, 25 * 256 :])
        nc.scalar.dma_start(
            out=wsb[:, :],
            in_=bass.AP(wt, 0, [[128, 128], [128 * 128, 36], [1, 128]]),
        )

        xin_pool = ctx.enter_context(tc.tile_pool(name="xin", bufs=3))
        sout_pool = ctx.enter_context(tc.tile_pool(name="sout", bufs=5))
        osts = const.tile([128, 64 * OSTRIDE], BF16, name="osts", tag="osts")
        mpsum_pool = ctx.enter_context(tc.tile_pool(name="mpsum", bufs=7, space="PSUM"))
        dpsum_pool = ctx.enter_context(tc.tile_pool(name="dpsum", bufs=1, space="PSUM"))

        # HAM warmup: staged dummy matmuls keep the PE busy through the
        # pipeline-fill window (deps fire every ~3us) so the first real
        # matmuls run at 2.4 GHz instead of paying the 1.2 GHz cold ramp
        dps = dpsum_pool.tile([128, 256], F32, name="dps", tag="dps")

        def warm_mms(n, rhs_ap):
            for _ in range(n):
                nc.tensor.matmul(
                    dps[:, :], lhsT=dtile[:, 0:128], rhs=rhs_ap,
                    start=True, stop=True,
                )

        # Outputs stage unstripped in one bf16 SBUF region and drain as 4
        # big contiguous out-DMAs issued after the loop: every DMA is
        # framework-serialized around DMA-transposes, so a PE-paced
        # out-DMA ahead of a transpose would throttle the whole supply
        # chain to PE pace.
        # tap order per group: the (dh=2, dw=2) taps get a 397-wide window
        # (the dropped psum col 397 contribution reads the zero pad column,
        # an exact no-op) and sit mid-sequence so start/stop matmuls are
        # full-width
        TAPS = [(0, t) for t in range(9)] + [(1, 8)] + [(1, t) for t in range(8)]

        def conv_group(b, g):
            pl = planes[b]
            for oct in range(2):
                mp = mpsum_pool.tile([128, NG], F32, name="mp", tag="mp")
                for i, (ct, tap) in enumerate(TAPS):
                    dh, dw = divmod(tap, 3)
                    q0 = ct * REG + (GR * g + dh) * ROWW + dw
                    widx = (ct * 9 + tap) * 2 + oct
                    n = NG - 1 if (dh == 2 and dw == 2) else NG
                    nc.tensor.matmul(
                        mp[:, :n],
                        lhsT=wsb[:, widx * 128 : (widx + 1) * 128],
                        rhs=pl[:, q0 : q0 + n],
                        start=(i == 0),
                        stop=(i == 17),
                    )
                oi = (b * 2 + oct) * 8 + g
                # evictions (PE-paced) live alone on the vector queue so
                # they never delay scatter-paced work on other queues
                nc.vector.tensor_copy(
                    osts[:, oi * OSTRIDE : oi * OSTRIDE + NG], mp[:, :]
                )

        # xin prefetch 2 calls ahead so transposes (which wait on the
        # scatter) never block the next input load on the sync FIFO
        xins = {}

        def load_xin(m):
            if m >= len(CALLS):
                return
            s, k = CALLS[m]
            kk = k * 256
            xin = xin_pool.tile([128, 7 * 256], BF16, name="xin", tag="xin")
            nc.sync.dma_start(
                out=xin[:, :kk],
                in_=bass.AP(xt, 128 * s * C, [[C, 128], [128 * C, k], [1, C]]),
            )
            xins[m] = xin

        def do_copies(mi):
            slot_i = (mi % STG_SLOTS) * 896
            for src_off, b_idx, pl_off, kind, n in segs[mi]:
                pl = planes[b_idx]
                for ct in range(2):
                    so = slot_i + src_off
                    po = ct * REG + pl_off
                    if kind == "rows":
                        src = stg[ct][:, so : so + 56 * n].rearrange(
                            "p (r x) -> p r x", r=n
                        )
                        dst = pl[:, po : po + n * ROWW].rearrange(
                            "p (r x) -> p r x", r=n
                        )[:, :, 0:56]
                        nc.scalar.copy(dst, src)
                    else:
                        nc.scalar.copy(
                            pl[:, po : po + n], stg[ct][:, so : so + n]
                        )

        load_xin(0)
        load_xin(1)
        warm_mms(20, dtile[:, 0:256])          # fires ~preamble end
        warm_mms(12, xins[0][:, 0:256])        # fires at xin0 load done
        warm_mms(10, xins[1][:, 0:256])        # fires at xin1 load done
        for m, (s, k) in enumerate(CALLS):
            kk = k * 256
            load_xin(m + 2)
            if m == 0:
                warm_mms(10, xins[2][:, 0:256])  # fires at xin2 load done
            xin = xins.pop(m)
            if m < 2:
                iap = idxh_sb[:, s * 256 : (s + k) * 256]
            else:
                sp = (s % PERIOD) * 256
                iap = idxs_sb[:, sp : sp + kk]
            sout = sout_pool.tile([128, 7 * 256], BF16, name="sout", tag="sout")
            nc.gpsimd.local_scatter(
                out_ap=sout[:, :kk],
                data_ap=xin[:, :kk],
                idxs_ap=iap,
                channels=128,
                num_elems=kk,
                num_idxs=kk,
            )
            slot = (m % STG_SLOTS) * 896
            for ct in range(2):
                dst = stg[ct][:, slot : slot + k * 128].rearrange(
                    "p (e l) -> p e l", e=k
                )
                nc.sync.dma_start_transpose(
                    dst, sout[:, ct * k * 128 : (ct + 1) * k * 128]
                )
            if m <= 1:
                # fires when this call's scatter completes (Pool sem, fast)
                warm_mms(10 if m == 0 else 8, sout[:, 0:256])
            # pad-copies on ACT (own SBUF port; DVE contends with GPSIMD);
            # the scalar queue carries ONLY these, so they are always
            # scatter-paced -- never queued behind a PE-paced eviction
            do_copies(m)
            # ready[0]'s conv issues after iteration 1's warmup phase so the
            # HAM bridge extends right up to the first real matmul
            if m == 1:
                for (b, g) in ready[0] + ready[1]:
                    conv_group(b, g)
            elif m > 1:
                for (b, g) in ready[m]:
                    conv_group(b, g)
        for b in range(B_LOC):
            for oct in range(2):
                last = b == B_LOC - 1 and oct == 1
                # per-group DMAs for the very last oct-block: the kernel-end
                # barrier then waits on a 51KB completion instead of 204KB
                step = 1 if last else 4
                for h in range(0, 8, step):
                    o0 = ((b * 2 + oct) * 8 + h) * OSTRIDE
                    d0 = (oct * 8 + h) * OSTRIDE
                    nc.sync.dma_start(
                        out=out[b, :, d0 : d0 + step * OSTRIDE],
                        in_=osts[:, o0 : o0 + step * OSTRIDE],
                    )

    nc.compile()
    return nc


def _host_prep(x, w, perm):
    import ml_dtypes

    # pixel-major bf16: [B, HW, C]
    xf = np.ascontiguousarray(
        x.reshape(B, C, HW).transpose(0, 2, 1)
    ).astype(ml_dtypes.bfloat16)

    wt = np.empty((36, 128, 128), dtype=ml_dtypes.bfloat16)
    wf = np.asarray(w, dtype=np.float32)
    for ct in range(2):
        for tap in range(9):
            kh, kw = divmod(tap, 3)
            for oct in range(2):
                i = (ct * 9 + tap) * 2 + oct
                wt[i] = wf[
                    oct * 128 : (oct + 1) * 128, ct * 128 : (ct + 1) * 128, kh, kw
                ].T.astype(ml_dtypes.bfloat16)

    iperm = np.empty((HW, C), dtype=np.int16)
    np.put_along_axis(
        iperm, perm.astype(np.int64), np.arange(C, dtype=np.int16)[None, :], axis=1
    )

    # steady idx table [128, PERIOD*256]: group g, partition p -> pixel
    # (128g+p) % HW; dst = ct_major(k=7): (ip>>7)*896 + (g%7)*128 + (ip&127)
    gg = np.arange(PERIOD)
    pp = np.arange(128)
    l = (128 * gg[:, None] + pp[None, :]) % HW          # [49, 128]
    ip = iperm[l].astype(np.int32)                      # [49, 128, 256]
    j = (gg % 7).astype(np.int32)[:, None, None]
    dst = (ip >> 7) * 896 + j * 128 + (ip & 127)
    idxs = np.ascontiguousarray(
        dst.astype(np.int16).transpose(1, 0, 2).reshape(128, PERIOD * 256)
    )

    # head table for the first 7 groups with call sizes (0,4),(4,3)
    idxh = np.empty((128, 7 * 256), dtype=np.int16)
    for (s, k) in CALLS[:2]:
        for g in range(s, s + k):
            lg = (128 * g + pp) % HW
            ipg = iperm[lg].astype(np.int32)            # [128, 256]
            d = (ipg >> 7) * (k * 128) + (g - s) * 128 + (ipg & 127)
            idxh[:, g * 256 : (g + 1) * 256] = d.astype(np.int16)

    in_maps = []
    for cidx in range(N_CORES):
        in_maps.append(
            {
                "xt": np.ascontiguousarray(
                    xf[cidx * B_LOC : (cidx + 1) * B_LOC].reshape(GPX, C)
                ),
                "wt": wt,
                "idxh": idxh,
                "idxs": idxs,
            }
        )
    return in_maps


def kernel(x, w, perm):
    global LAST_RESULT
    _install_ntff_shim()
    from concourse.bass_utils import run_bass_kernel_spmd

    x = np.asarray(x, dtype=np.float32)
    w = np.asarray(w, dtype=np.float32)
    perm = np.asarray(perm)

    if "nc" not in _STATE:
        _STATE["nc"] = _build_kernel()
    nc = _STATE["nc"]

    in_maps = _host_prep(x, w, perm)
    res = run_bass_kernel_spmd(nc, in_maps, core_ids=list(range(N_CORES)))
    LAST_RESULT = res
    outs = []
    for r in res.results:
        # [b, p, (oct*8+g)*399 + r*57 + x] -> [b, oct*128+p, g*392 + r*56 + x]
        a = np.asarray(r["out"], dtype=np.float32).reshape(
            B_LOC, 128, 2, 8, GR, ROWW
        )[..., :56]
        a = a.transpose(0, 2, 1, 3, 4, 5).reshape(B_LOC, C, H, W)
        outs.append(a)
    return np.ascontiguousarray(np.concatenate(outs, axis=0), dtype=np.float32)
